# revision 1
# baseline (speedup 1.0000x reference)
"""BitNetAttention Trainium2 kernel — 8-core SPMD, query-sharded, collective-free.

Per core c: batch b = c//4, query rows 512*(c%4)..+512. The host hands each
core its batch's FULL hidden states rotated so the core's own 512 query rows
sit at rows 0:511 (softmax is key-permutation invariant, so rotating the key
axis is harmless). Each core int4-quantizes all 2048 rows (redundant across
the 4-core batch group — cheaper than any collective), computes q for its 512
queries and k/v for all 2048 keys with exact fp8e4m3 DoubleRow matmuls
(int4 values and ternary weights are exact in fp8), interleaving projections
with quantization per 4-tile key slice. Attention: scoresT layout, exp on ACT
to float32r, PV matmul with a 65th all-ones lhsT column yielding the softmax
denominator Z exactly. Tail: int8-quant + top-50% sparsify (bisection on
integer levels, interleaved across the 4 q-tiles to hide latency) + local
o-projection. No collectives anywhere. Host reassembles transposed shards.
"""
import sys
import math

sys.path.insert(0, "/opt/trn_rl_repo")

import numpy as np

B, S, H, NH = 2, 2048, 1024, 16
HD = H // NH          # 64
SHARD = 512           # query rows per core
NCORES = 8
SQRT7 = math.sqrt(7.0)
MAGIC = float(np.float32(3 * 2**22))  # 1.5 * 2^23: RNE rounding magic

_cache = {}


def _build():
    import concourse.bass as bass
    import concourse.bacc as bacc
    import concourse.mybir as mybir
    from concourse.tile import TileContext
    from concourse.masks import make_identity

    dt = mybir.dt
    Alu = mybir.AluOpType
    Act = mybir.ActivationFunctionType
    X = mybir.AxisListType.X
    DR = mybir.MatmulPerfMode.DoubleRow

    nc = bacc.Bacc("TRN2", target_bir_lowering=False, debug=False,
                   num_devices=NCORES)

    hs_in = nc.dram_tensor("hs", [S, H], dt.float32, kind="ExternalInput")
    wq8_in = nc.dram_tensor("wq8", [H, H], dt.float8e4, kind="ExternalInput")
    wk8_in = nc.dram_tensor("wk8", [H, H], dt.float8e4, kind="ExternalInput")
    wv8_in = nc.dram_tensor("wv8", [H, H], dt.float8e4, kind="ExternalInput")
    woT_in = nc.dram_tensor("woT", [H, H], dt.float32r, kind="ExternalInput")
    cst_in = nc.dram_tensor("cst", [8], dt.float32, kind="ExternalInput")
    outT_out = nc.dram_tensor("outT", [H, SHARD], dt.float32, kind="ExternalOutput")

    brow = nc.dram_tensor("brow", [S], dt.float32)
    srow = nc.dram_tensor("srow", [SHARD], dt.float32)
    ctxd = nc.dram_tensor("ctxd", [SHARD, H], dt.float32)
    qTd = nc.dram_tensor("qTd", [H, SHARD], dt.float32r)

    NT = S // 128       # 16 s-tiles (all tokens)
    QT = SHARD // 128   # 4 q-tiles (own queries)
    HT = H // 128       # 8 h/f/c-tiles
    KT = S // 128       # 16 k-tiles

    with TileContext(nc) as tc:
        with tc.tile_pool(name="base", bufs=1) as bp, \
             tc.tile_pool(name="work", bufs=2) as wp, \
             tc.tile_pool(name="mmps", bufs=2, space="PSUM") as pmm:

            ident = bp.tile([128, 128], dt.float32)
            make_identity(nc, ident[:])
            identb = bp.tile([128, 128], dt.bfloat16)
            nc.vector.tensor_copy(identb[:], ident[:])
            ones_row = bp.tile([1, 128], dt.float32)
            nc.vector.memset(ones_row[:], 1.0)

            cst_sb = bp.tile([1, 8], dt.float32)
            nc.sync.dma_start(out=cst_sb[:], in_=cst_in[None, :])
            ps_c = pmm.tile([128, 512], dt.float32, tag="mm")
            nc.tensor.matmul(ps_c[:, 0:8], ones_row[:], cst_sb[:], start=True, stop=True)
            cst_bc = bp.tile([128, 8], dt.float32)
            nc.vector.tensor_copy(cst_bc[:], ps_c[:, 0:8])
            AQ8 = cst_bc[:, 0:1]
            AK = cst_bc[:, 1:2]
            AV = cst_bc[:, 2:3]
            AO127 = cst_bc[:, 3:4]

            av_cols = bp.tile([128, NT], dt.float32)

            with tc.tile_pool(name="kv", bufs=1) as kvp:
                kT = kvp.tile([128, HT, S], dt.float32r)
                vres = kvp.tile([128, KT, NH, HD + 1], dt.float32r)

                # ====== merged phase 1-3: quant + projections, pipelined ====
                with tc.tile_pool(name="xq", bufs=1) as xp, \
                     tc.tile_pool(name="wts", bufs=1) as wtp, \
                     tc.tile_pool(name="scl", bufs=1) as sp, \
                     tc.tile_pool(name="qtmp", bufs=1) as qp, \
                     tc.tile_pool(name="prps", bufs=4, space="PSUM") as prps, \
                     tc.tile_pool(name="tps8", bufs=2, space="PSUM") as tpp:
                    xqT8 = xp.tile([128, HT, S], dt.float8e4)
                    wq8 = wtp.tile([128, HT, H], dt.float8e4)
                    wk8 = wtp.tile([128, HT, H], dt.float8e4)
                    wv8 = wtp.tile([128, HT, H], dt.float8e4)
                    aq_bc = sp.tile([128, SHARD], dt.float32)
                    ak_bc = sp.tile([128, S], dt.float32)

                    for i in range(NT):
                        hst = qp.tile([128, H], dt.float32, tag="hs", bufs=2)
                        nc.sync.dma_start(out=hst[:], in_=hs_in[i * 128:(i + 1) * 128, :])
                        # weight loads deferred behind the first hidden-state
                        # tiles: not needed until the i==3 projection burst
                        if i == 0:
                            nc.sync.dma_start(out=wq8[:], in_=wq8_in.rearrange("(a p) f -> p a f", p=128))
                        elif i == 1:
                            nc.sync.dma_start(out=wk8[:], in_=wk8_in.rearrange("(a p) f -> p a f", p=128))
                        elif i == 2:
                            nc.sync.dma_start(out=wv8[:], in_=wv8_in.rearrange("(a p) f -> p a f", p=128))
                        ytile = qp.tile([128, H], dt.float32, tag="yt", bufs=2)
                        ssum = wp.tile([128, 1], dt.float32)
                        nc.scalar.activation(ytile[:], hst[:], Act.Abs,
                                             accum_out=ssum[:])
                        beta = wp.tile([128, 1], dt.float32)
                        nc.vector.tensor_scalar(out=beta[:], in0=ssum[:],
                                                scalar1=float(np.float32(1.0 / H)),
                                                scalar2=None, op0=Alu.mult)
                        nc.sync.dma_start(out=brow[i * 128:(i + 1) * 128], in_=beta[:, 0])
                        denom = wp.tile([128, 1], dt.float32)
                        nc.vector.tensor_scalar(out=denom[:], in0=beta[:],
                                                scalar1=float(np.float32(1e-5)),
                                                scalar2=None, op0=Alu.add)
                        nc.vector.tensor_scalar(out=av_cols[:, i:i + 1], in0=beta[:],
                                                scalar1=AV, scalar2=None, op0=Alu.mult)
                        r2 = wp.tile([128, 1], dt.float32)
                        nc.vector.reciprocal(r2[:], denom[:])
                        nc.vector.tensor_scalar(out=ytile[:], in0=hst[:],
                                                scalar1=r2[:],
                                                scalar2=float(np.float32(SQRT7)),
                                                op0=Alu.mult, op1=Alu.mult)
                        # round to nearest-even integer; values beyond +-16
                        # round coarsely in fp8 but clip to [-8,7] later anyway
                        xqb = qp.tile([128, H], dt.bfloat16, tag="xqb", bufs=2)
                        nc.gpsimd.tensor_scalar(out=xqb[:], in0=ytile[:],
                                                scalar1=MAGIC, scalar2=MAGIC,
                                                op0=Alu.add, op1=Alu.subtract)
                        tp8 = tpp.tile([128, H], dt.bfloat16, tag="tp8")
                        for jt in range(HT):
                            nc.tensor.transpose(tp8[:, jt * 128:(jt + 1) * 128],
                                                xqb[:, jt * 128:(jt + 1) * 128],
                                                identb[:])
                        nc.vector.tensor_scalar(
                            out=xqT8[:, :, i * 128:(i + 1) * 128],
                            in0=tp8[:].rearrange("p (a q) -> p a q", q=128),
                            scalar1=float(np.float32(-8.0)),
                            scalar2=float(np.float32(7.0)),
                            op0=Alu.max, op1=Alu.min)

                        if i % 4 != 3:
                            continue
                        ks = i // 4
                        # ---- per-slice scale row: keys ks*512..+512 --------
                        beta_row = wp.tile([1, 512], dt.float32, tag="brw", bufs=1)
                        nc.sync.dma_start(out=beta_row[:],
                                          in_=brow[None, ks * 512:(ks + 1) * 512])
                        ps_a = pmm.tile([128, 512], dt.float32, tag="mm")
                        nc.tensor.matmul(ps_a[:], ones_row[:], beta_row[:],
                                         start=True, stop=True)
                        nc.vector.tensor_scalar(
                            out=ak_bc[:, ks * 512:(ks + 1) * 512],
                            in0=ps_a[:], scalar1=AK, scalar2=None, op0=Alu.mult)
                        if ks == 0:
                            nc.vector.tensor_scalar(out=aq_bc[:], in0=ps_a[:],
                                                    scalar1=AQ8, scalar2=None,
                                                    op0=Alu.mult)
                            # ---- q projection (own 512 rows) --------------
                            for ft in range(HT):
                                ps = prps.tile([128, 512], dt.float32, tag="pj")
                                for hp in range(4):
                                    nc.tensor.matmul(ps[:],
                                                     wq8[:, 2 * hp:2 * hp + 2, ft * 128:(ft + 1) * 128],
                                                     xqT8[:, 2 * hp:2 * hp + 2, 0:SHARD],
                                                     start=(hp == 0), stop=(hp == 3),
                                                     perf_mode=DR)
                                qsl = qp.tile([128, SHARD], dt.float32r, tag="qsl", bufs=2)
                                nc.vector.tensor_tensor(out=qsl[:], in0=ps[:],
                                                        in1=aq_bc[:], op=Alu.mult)
                                nc.sync.dma_start(
                                    out=qTd[ft * 128:(ft + 1) * 128, :], in_=qsl[:])
                        # ---- k projection for this key slice ---------------
                        for ft in range(HT):
                            ps = prps.tile([128, 512], dt.float32, tag="pj")
                            for hp in range(4):
                                nc.tensor.matmul(ps[:],
                                                 wk8[:, 2 * hp:2 * hp + 2, ft * 128:(ft + 1) * 128],
                                                 xqT8[:, 2 * hp:2 * hp + 2, ks * 512:(ks + 1) * 512],
                                                 start=(hp == 0), stop=(hp == 3),
                                                 perf_mode=DR)
                            nc.vector.tensor_tensor(
                                out=kT[:, ft, ks * 512:(ks + 1) * 512], in0=ps[:],
                                in1=ak_bc[:, ks * 512:(ks + 1) * 512], op=Alu.mult)
                        # ---- v projection for k-tiles of this slice --------
                        for kt in range(4 * ks, 4 * ks + 4):
                            for fc in range(2):
                                ps = prps.tile([128, 512], dt.float32, tag="pj")
                                for hp in range(4):
                                    nc.tensor.matmul(ps[:],
                                                     xqT8[:, 2 * hp:2 * hp + 2, kt * 128:(kt + 1) * 128],
                                                     wv8[:, 2 * hp:2 * hp + 2, fc * 512:(fc + 1) * 512],
                                                     start=(hp == 0), stop=(hp == 3),
                                                     perf_mode=DR)
                                nc.scalar.activation(
                                    vres[:, kt, fc * 8:(fc + 1) * 8, 0:HD],
                                    ps[:].rearrange("p (h d) -> p h d", d=HD),
                                    Act.Copy, scale=av_cols[:, kt:kt + 1])

                ones_f = wp.tile([128, NH], dt.float32, tag="onesf")
                nc.vector.memset(ones_f[:], 1.0)
                ones_r = wp.tile([128, NH], dt.float32r, tag="onesr")
                nc.vector.tensor_copy(ones_r[:], ones_f[:])
                for t in range(KT):
                    nc.vector.tensor_copy(
                        vres[:, t, :, HD:HD + 1],
                        ones_r.rearrange("p (h o) -> p h o", o=1))

                # ===== phase 4+5: attention by query-halves; the tail of
                # half A (int8 quant + topk bisection, DVE-heavy) runs
                # concurrently with attention of half B ======================
                QW = 256
                with tc.tile_pool(name="tailp", bufs=1) as tlp:
                    nm = tlp.tile([128, QT, H], dt.bfloat16)

                    def emit_tail_half(qh):
                        sts = (2 * qh, 2 * qh + 1)
                        cxs2, nbs2, abs2, junks2 = [], [], [], []
                        for s2, st in enumerate(sts):
                            cx = tlp.tile([128, H], dt.float32, tag="cx", bufs=2)
                            nc.sync.dma_start(out=cx[:],
                                              in_=ctxd[st * 128:(st + 1) * 128, :])
                            cxs2.append(cx)
                        for s2, st in enumerate(sts):
                            cx = cxs2[s2]
                            gmax = wp.tile([128, 1], dt.float32, tag=f"gm{s2}")
                            nc.vector.tensor_reduce(gmax[:], cx[:], axis=X, op=Alu.max,
                                                    apply_absolute_value=True)
                            gmax = gmax[:]
                            gd = wp.tile([128, 1], dt.float32, tag=f"gd{s2}")
                            nc.vector.tensor_scalar(out=gd[:], in0=gmax,
                                                    scalar1=float(np.float32(1e-5)),
                                                    scalar2=None, op0=Alu.add)
                            rg = wp.tile([128, 1], dt.float32, tag=f"rg{s2}")
                            nc.vector.reciprocal(rg[:], gd[:])
                            sc = wp.tile([128, 1], dt.float32, tag=f"sc{s2}")
                            nc.vector.tensor_scalar(out=sc[:], in0=gmax,
                                                    scalar1=AO127,
                                                    scalar2=None, op0=Alu.mult)
                            nc.sync.dma_start(out=srow[st * 128:(st + 1) * 128],
                                              in_=sc[:, 0])
                            y = tlp.tile([128, H], dt.float32, tag="y", bufs=2)
                            nc.vector.tensor_scalar(out=y[:], in0=cx[:],
                                                    scalar1=rg[:],
                                                    scalar2=float(np.float32(127.0)),
                                                    op0=Alu.mult, op1=Alu.mult)
                            eng = nc.vector if s2 == 0 else nc.gpsimd
                            nb = tlp.tile([128, H], dt.bfloat16, tag="nb", bufs=2)
                            eng.tensor_scalar(out=nb[:], in0=y[:], scalar1=MAGIC,
                                              scalar2=MAGIC, op0=Alu.add,
                                              op1=Alu.subtract)
                            ab = tlp.tile([128, H], dt.bfloat16, tag="ab", bufs=2)
                            nc.vector.scalar_tensor_tensor(out=ab[:], in0=nb[:],
                                                           scalar=-1.0, in1=nb[:],
                                                           op0=Alu.mult, op1=Alu.max)
                            junk = tlp.tile([128, H], dt.bfloat16, tag="junk", bufs=2)
                            nbs2.append(nb); abs2.append(ab); junks2.append(junk)
                        lo2 = wp.tile([128, 2], dt.float32, tag="lo2")
                        hi2 = wp.tile([128, 2], dt.float32, tag="hi2")
                        mid2 = wp.tile([128, 2], dt.float32, tag="mid2")
                        cnt2 = wp.tile([128, 2], dt.float32, tag="cnt2")
                        tk2 = wp.tile([128, 2], dt.uint32, tag="tk2")
                        nc.vector.memset(lo2[:], -1.0)
                        nc.vector.memset(hi2[:], 128.0)
                        for it in range(8):
                            nc.vector.tensor_tensor(out=mid2[:], in0=lo2[:],
                                                    in1=hi2[:], op=Alu.add)
                            nc.vector.tensor_scalar(out=mid2[:], in0=mid2[:],
                                                    scalar1=float(np.float32(0.5)),
                                                    scalar2=float(np.float32(-0.25)),
                                                    op0=Alu.mult, op1=Alu.add)
                            nc.vector.tensor_scalar(out=mid2[:], in0=mid2[:],
                                                    scalar1=MAGIC, scalar2=MAGIC,
                                                    op0=Alu.add, op1=Alu.subtract)
                            for s2 in range(2):
                                nc.vector.scalar_tensor_tensor(
                                    out=junks2[s2][:], in0=abs2[s2][:],
                                    scalar=mid2[:, s2:s2 + 1], in1=abs2[s2][:],
                                    op0=Alu.is_le, op1=Alu.bypass,
                                    accum_out=cnt2[:, s2:s2 + 1])
                            nc.vector.tensor_scalar(out=tk2[:], in0=cnt2[:],
                                                    scalar1=float(np.float32(512.0)),
                                                    scalar2=None, op0=Alu.is_ge)
                            nc.vector.copy_predicated(hi2[:], tk2[:], mid2[:])
                            nc.vector.tensor_scalar(out=tk2[:], in0=cnt2[:],
                                                    scalar1=float(np.float32(512.0)),
                                                    scalar2=None, op0=Alu.is_lt)
                            nc.vector.copy_predicated(lo2[:], tk2[:], mid2[:])
                        for s2, st in enumerate(sts):
                            nc.vector.scalar_tensor_tensor(
                                out=nm[:, st, :], in0=abs2[s2][:],
                                scalar=hi2[:, s2:s2 + 1], in1=nbs2[s2][:],
                                op0=Alu.is_ge, op1=Alu.mult)

                    with tc.tile_pool(name="scps", bufs=2, space="PSUM") as psc, \
                         tc.tile_pool(name="ctxps", bufs=2, space="PSUM") as pcx, \
                         tc.tile_pool(name="probs", bufs=4) as prp, \
                         tc.tile_pool(name="cwork", bufs=2) as cwp:
                        for qh in range(2):
                            qlo = qh * QW
                            for pr in range(NH // 2):
                                hA, hB = 2 * pr, 2 * pr + 1
                                qTs = cwp.tile([128, QW], dt.float32r, tag="qts")
                                nc.sync.dma_start(
                                    out=qTs[:],
                                    in_=qTd[pr * 128:(pr + 1) * 128, qlo:qlo + QW])
                                pcA = pcx.tile([HD + 1, QW], dt.float32, tag="ctx")
                                pcB = pcx.tile([HD + 1, QW], dt.float32, tag="ctx")
                                for g in range(KT // 4):
                                    psA = psc.tile([128, 1024], dt.float32, tag="sc")
                                    psB = psc.tile([128, 1024], dt.float32, tag="sc")
                                    for gi in range(4):
                                        t = 4 * g + gi
                                        ksl = kT[:, pr, t * 128:(t + 1) * 128]
                                        nc.tensor.matmul(psA[:, gi * QW:(gi + 1) * QW],
                                                         ksl[0:64, :], qTs[0:64, :],
                                                         start=True, stop=True,
                                                         tile_position=(0, 0))
                                        nc.tensor.matmul(psB[:, gi * QW:(gi + 1) * QW],
                                                         ksl[64:128, :], qTs[64:128, :],
                                                         start=True, stop=True,
                                                         tile_position=(64, 0))
                                    pbA = prp.tile([128, 1024], dt.float32r, tag="pb")
                                    pbB = prp.tile([128, 1024], dt.float32r, tag="pb")
                                    nc.scalar.activation(pbA[:], psA[:], Act.Exp)
                                    nc.scalar.activation(pbB[:], psB[:], Act.Exp)
                                    for gi in range(4):
                                        t = 4 * g + gi
                                        nc.tensor.matmul(pcA[:], vres[:, t, hA, :],
                                                         pbA[:, gi * QW:(gi + 1) * QW],
                                                         start=(t == 0), stop=(t == KT - 1))
                                        nc.tensor.matmul(pcB[:], vres[:, t, hB, :],
                                                         pbB[:, gi * QW:(gi + 1) * QW],
                                                         start=(t == 0), stop=(t == KT - 1))
                                csbA = cwp.tile([HD + 1, QW], dt.float32, tag="csbA")
                                nc.vector.tensor_copy(csbA[:], pcA[:])
                                csbB = cwp.tile([HD + 1, QW], dt.float32, tag="csbB")
                                nc.vector.tensor_copy(csbB[:], pcB[:])
                                for st2 in range(2):
                                    st = 2 * qh + st2
                                    cslc = cwp.tile([128, 2, HD], dt.float32,
                                                    tag="cslc", bufs=3)
                                    for hx, csb in ((0, csbA), (1, csbB)):
                                        pt = pmm.tile([128, 512], dt.float32, tag="mm")
                                        nc.tensor.transpose(
                                            pt[:, 0:HD + 1],
                                            csb[:, st2 * 128:(st2 + 1) * 128],
                                            ident[0:HD + 1, 0:HD + 1])
                                        rz = wp.tile([128, 1], dt.float32)
                                        nc.vector.reciprocal(rz[:], pt[:, HD:HD + 1])
                                        nc.vector.tensor_scalar(
                                            out=cslc[:, hx, :],
                                            in0=pt[:, 0:HD], scalar1=rz[:],
                                            scalar2=None, op0=Alu.mult)
                                    nc.sync.dma_start(
                                        out=ctxd[st * 128:(st + 1) * 128,
                                                 hA * HD:(hA + 2) * HD],
                                        in_=cslc[:])
                            # tail of this half overlaps the next half's
                            # attention (or runs last for qh=1)
                            emit_tail_half(qh)

                    # ========== phase 6: transpose + scale + o_proj ==========
                    with tc.tile_pool(name="ph6", bufs=1) as p6, \
                         tc.tile_pool(name="tpbp", bufs=4, space="PSUM") as tbp:
                        sc_row = p6.tile([1, SHARD], dt.float32)
                        nc.sync.dma_start(out=sc_row[:], in_=srow[None, :])
                        ps_s = pmm.tile([128, 512], dt.float32, tag="mm")
                        nc.tensor.matmul(ps_s[:], ones_row[:], sc_row[:],
                                         start=True, stop=True)
                        sc_bc = p6.tile([128, SHARD], dt.float32)
                        nc.scalar.activation(sc_bc[:], ps_s[:], Act.Copy)

                        rhsT = p6.tile([128, HT, SHARD], dt.float32r)
                        for st in range(QT):
                            for ct in range(HT):
                                tpb = tbp.tile([128, 128], dt.bfloat16, tag="tpb")
                                nc.tensor.transpose(tpb[:],
                                                    nm[:, st, ct * 128:(ct + 1) * 128],
                                                    identb[:])
                                nc.vector.tensor_tensor(
                                    out=rhsT[:, ct, st * 128:(st + 1) * 128],
                                    in0=tpb[:],
                                    in1=sc_bc[:, st * 128:(st + 1) * 128],
                                    op=Alu.mult)

                        for ft in range(HT):
                            wsl = p6.tile([128, HT, 128], dt.float32r, tag="wsl", bufs=3)
                            nc.sync.dma_start(
                                out=wsl[:],
                                in_=woT_in[:, ft * 128:(ft + 1) * 128]
                                    .rearrange("(a p) f -> p a f", p=128))
                            ps = pmm.tile([128, 512], dt.float32, tag="mm")
                            for ct in range(HT):
                                nc.tensor.matmul(ps[:], wsl[:, ct, :],
                                                 rhsT[:, ct, :], start=(ct == 0),
                                                 stop=(ct == HT - 1))
                            ot = p6.tile([128, 512], dt.float32, tag="ot", bufs=2)
                            nc.scalar.activation(ot[:], ps[:], Act.Copy)
                            nc.sync.dma_start(out=outT_out[ft * 128:(ft + 1) * 128, :],
                                              in_=ot[:])

    nc.compile()
    return nc


def kernel(hidden_states, Wq, Wk, Wv, Wo, sq, sk, sv, so):
    import jax
    import jax.numpy as jnp
    from concourse.bass_utils import run_bass_kernel_spmd
    import ml_dtypes

    cpu = jax.devices("cpu")[0]

    def wquant(W, s):
        with jax.default_device(cpu):
            W32 = np.asarray(W, np.float32)
            w_mean = jnp.mean(jnp.abs(jnp.asarray(W32)))
            w_q = jnp.clip(jnp.round(jnp.asarray(W32) / (w_mean + 1e-5)), -1.0, 1.0)
            return np.asarray(w_q, np.float32), np.float32(np.float32(w_mean) * np.float32(s))

    hidden_states = np.ascontiguousarray(np.asarray(hidden_states, np.float32))
    wq_q, aq = wquant(Wq, np.asarray(sq).reshape(-1)[0])
    wk_q, ak = wquant(Wk, np.asarray(sk).reshape(-1)[0])
    wv_q, av = wquant(Wv, np.asarray(sv).reshape(-1)[0])
    wo_q, ao = wquant(Wo, np.asarray(so).reshape(-1)[0])

    wq8 = np.ascontiguousarray(wq_q.T).astype(ml_dtypes.float8_e4m3)
    wk8 = np.ascontiguousarray(wk_q.T).astype(ml_dtypes.float8_e4m3)
    wv8 = np.ascontiguousarray(wv_q.T).astype(ml_dtypes.float8_e4m3)
    woT = np.ascontiguousarray(wo_q.T).astype(np.float32)

    cst = np.zeros(8, np.float32)
    cst[0] = np.float32(aq / np.float32(math.sqrt(HD)))
    cst[1] = ak
    cst[2] = av
    cst[3] = np.float32(ao / np.float32(127.0))

    if "nc" not in _cache:
        _cache["nc"] = _build()
    nc = _cache["nc"]

    in_maps = []
    for c in range(NCORES):
        b, j = c // 4, c % 4
        hs_rot = np.ascontiguousarray(np.roll(hidden_states[b], -j * SHARD, axis=0))
        in_maps.append({
            "hs": hs_rot,
            "wq8": wq8, "wk8": wk8, "wv8": wv8, "woT": woT, "cst": cst,
        })

    _cache["last_in_maps"] = in_maps
    res = run_bass_kernel_spmd(nc, in_maps, list(range(NCORES)))
    _cache["last_res"] = res
    out = np.empty((B, S, H), np.float32)
    for c in range(NCORES):
        b, j = c // 4, c % 4
        out[b, j * SHARD:(j + 1) * SHARD, :] = res.results[c]["outT"].T
    return out

